# revision 54
# baseline (speedup 1.0000x reference)
"""Multi-head attention kernel for 8 TRN2 NeuronCores.

Problem: b=2, n=2048, d=1024, heads=16, hd=64.
  q/k/v = x @ W{q,k,v}.T (+ zero bias)
  per head: softmax(q k^T / sqrt(d)) @ v
  out = concat @ Wo.T (+ zero bias)

Sharding (8 cores): data-parallel over batch (2) x tensor-parallel over
heads (16 heads -> 4 groups of 4). Core c handles batch c//4, heads
4*(c%4) .. 4*(c%4)+3 (feature slice of 256 columns). Wo is applied
row-parallel: each core emits a partial output; the host sums the 4
partials per batch (and untransposes). No collectives needed.

All matmuls run in float32r (TF32-like: ~1.5e-4 rel err on a K=1024
contraction, 4x the fp32 PE rate, full rate only when the moving free
dim is >=256). Raw fp32 bits are DMA'd directly into f32r tiles
(measured identical to an explicit rounding pass). On-chip f32r
operands (Q^T/K^T/V/P^T/out^T) are written by rounding copy/activation
producers as the walrus verifier requires.

Key structure decisions (all measured on HW):
 - everything is pre-transposed on the host so the kernel needs zero
   on-device transposes: xT (d,n), wqT/wkT/wvT (d,256), woT (256,d).
 - Q^T/K^T [feat, n] via dc-outer accumulation streaming behind the
   xT DMA; V in natural [n, feat] layout with a ones column appended
   (the ones column accumulates the softmax denominators during AV).
 - K^T is stored zero-padded per head to a full 128-row stationary:
   K=64 matmuls run at 2 cyc/row and read as low PE activity (HAM
   clock-gates to half speed); zero-padded K=128 runs at 1 cyc/row.
 - scores^T[k, q] (PE) -> exp via ScalarE reading 2 PSUM banks per
   call (the ACT engine is the pacing floor: n*n*heads/core exps at 1
   elem/cycle/lane) -> AV accumulates V_aug^T . P^T in PSUM [65, q].
 - passes are (q-half, head)-ordered and their emission is interleaved
   with the fc=1 projections so the PE fills ACT-paced slack; each
   q-half's output projection runs in the next half's ACT shadow.
 - normalize: copy avo out of PSUM fast (frees the accumulator), then
   reciprocal in a [128, 8] partition-scattered layout (a [1, 1024]
   row reciprocal is single-lane and 60x slower), partition_broadcast
   on GpSimd, multiply on DVE.
 - output projection keeps woT stationary (2 moving blocks per weight
   load) and emits the partial TRANSPOSED [d, n]; the host untransposes.

Biases are structurally zero in this problem spec and are skipped.
"""

import numpy as np

HEADS = 16
D = 1024
N = 2048
B = 2
N_CORES = 8
HPC = HEADS // (N_CORES // B)  # heads per core = 4
HD = D // HEADS                # 64
F = HPC * HD                   # 256 features per core
P = 128


def build_nc(n=N, d=D, hpc=HPC, hd=HD):
    """Build the per-core Bass program (SPMD: same program on all 8 cores)."""
    import concourse.bass as bass
    import concourse.tile as tile
    from concourse import bacc, mybir

    f32 = mybir.dt.float32
    bf16 = mybir.dt.bfloat16
    f = hpc * hd            # per-core feature count (256)
    FC = f // P             # feature chunks (2)
    DC = d // P             # contraction chunks over d (8)
    NT = n // P             # n tiles / k chunks (16)
    QB = min(512, n)        # matmul moving block
    SCW = min(1024, n)      # scores psum width (2 banks)
    NSC = n // SCW          # q-halves
    scale = 1.0 / float(np.sqrt(np.float32(d)))

    nc = bacc.Bacc("TRN2")

    xT = nc.declare_dram_parameter("xT", [d, n], bf16, isOutput=False)
    wqT = nc.declare_dram_parameter("wqT", [d, f], bf16, isOutput=False)
    wkT = nc.declare_dram_parameter("wkT", [d, f], bf16, isOutput=False)
    wvT = nc.declare_dram_parameter("wvT", [d, f], bf16, isOutput=False)
    woT = nc.declare_dram_parameter("woT", [f, d], bf16, isOutput=False)
    out = nc.declare_dram_parameter("out", [d, n], f32, isOutput=True)

    xT_c = xT.rearrange("(c p) n -> c p n", p=P)
    wqT_c = wqT.rearrange("(c p) f -> c p f", p=P)
    wkT_p = wkT.rearrange("(c p) f -> p c f", p=P)
    wvT_p = wvT.rearrange("(c p) f -> p c f", p=P)
    woT_p = woT.rearrange("(c p) n -> p c n", p=P)

    with tile.TileContext(nc) as tc:
        with (
            tc.tile_pool(name="qkv", bufs=1) as qkv,
            tc.tile_pool(name="outT", bufs=1) as outp,
            tc.tile_pool(name="wosb", bufs=4) as wosbp,
            # phase-2 pools created before the phase-1 pools so their
            # SBUF/PSUM ranges are disjoint: early heads' attention overlaps
            # the fc=1 projections with no pool-reuse serialization
            tc.tile_pool(name="pt", bufs=2) as ptp,
            tc.tile_pool(name="norm", bufs=2) as normp,
            tc.tile_pool(name="scps", bufs=2, space="PSUM") as scps,
            tc.tile_pool(name="avps", bufs=1, space="PSUM") as avps,
        ):
            QT_sb = qkv.tile([P, FC, n], bf16)
            # per-head K^T, zero-padded to a full 128-row stationary (head h
            # occupies partition rows po..po+hd, matching its rows in QT)
            KTz_sb = qkv.tile([P, hpc, n], bf16)
            V_sb = qkv.tile([P, NT, hpc, hd + 1], bf16)
            outT_sb = outp.tile([P, FC, n], bf16)
            # ones column of V_aug via broadcast copy
            ones_c = outp.tile([P, 1], f32)
            nc.vector.memset(ones_c[:], 1.0)
            nc.vector.tensor_copy(
                V_sb[:, :, :, hd : hd + 1],
                ones_c.to_broadcast([P, NT, hpc, 1]),
            )
            def fill_ktz_zeros():
                # KTz zero padding: only the COMPLEMENT rows of each head's
                # stationary stay zero (head 2fc occupies rows 0:hd, head
                # 2fc+1 rows hd:2hd). Zeroing just those keeps the fill
                # disjoint from the K^T copies (no WAR serialization); it
                # runs on the scalar engine, emitted AFTER the scalar-queue
                # DMA issues so it doesn't delay them.
                nc.scalar.memzero(KTz_sb[hd : 2 * hd, 0:hpc:2, :])
                nc.scalar.memzero(KTz_sb[0:hd, 1:hpc:2, :])

            def pass_begin():
                return avps.tile([hd + 1, SCW], f32, tag="avo", name="avo")

            def pass_blocks(avo, h, sh, kcs, mid_kc=None):
                """scores^T -> exp -> AV accumulate for k-chunks `kcs`.
                mid_kc (e.g. the V tile build feeding this kc's AV) emits
                BETWEEN scores and AV so the exp starts as soon as the
                scores land instead of queueing behind the filler's PE
                work."""
                fc = (h * hd) // P
                q0 = sh * SCW
                for kc in kcs:
                    sc = scps.tile([P, SCW], f32, tag="sc")
                    for qc in range(SCW // QB):
                        nc.tensor.matmul(
                            sc[:, qc * QB : (qc + 1) * QB],
                            KTz_sb[:, h, kc * P : (kc + 1) * P],
                            QT_sb[:, fc, q0 + qc * QB : q0 + (qc + 1) * QB],
                            start=True,
                            stop=True,
                        )
                    pt = ptp.tile([P, SCW], bf16, tag="pt")
                    nc.scalar.activation(
                        pt[:], sc[:], mybir.ActivationFunctionType.Exp,
                        scale=scale,
                    )
                    if mid_kc is not None:
                        mid_kc(kc)
                    for qc in range(SCW // QB):
                        nc.tensor.matmul(
                            avo[:, qc * QB : (qc + 1) * QB],
                            V_sb[:, kc, h, :],
                            pt[:, qc * QB : (qc + 1) * QB],
                            start=(kc == 0),
                            stop=(kc == NT - 1),
                        )

            def act_reciprocal(out_ap, in_ap):
                """ACT-engine reciprocal, bypassing the bass API's accuracy
                guard (tolerance here is 2e-2; the ACT table recip's error is
                orders below that). Single instruction, no partition scatter
                needed — ACT paces on free size, not partition count."""
                return nc.scalar.add_instruction(
                    mybir.InstActivation(
                        name=nc.get_next_instruction_name(),
                        func=mybir.ActivationFunctionType.Reciprocal,
                        ins=[
                            nc.scalar.lower_ap(in_ap),
                            mybir.ImmediateValue(
                                dtype=mybir.dt.float32, value=0.0
                            ),
                            mybir.ImmediateValue(
                                dtype=mybir.dt.float32, value=1.0
                            ),
                            mybir.ImmediateValue(
                                dtype=mybir.dt.float32, value=0.0
                            ),
                        ],
                        outs=[nc.scalar.lower_ap(out_ap)],
                    )
                )

            def pass_end_q(avo, h, sh, c0, cw, tail=False):
                """Normalize columns [c0, c0+cw) of avo: rows 0..hd-1 divided
                by row hd (the softmax sums). The avo drain copy runs on the
                otherwise-idle gpsimd so it never queues behind DVE work
                (stalling the next pass's first AV on the avo banks).
                reciprocal is single-lane-slow on a [1, cw] row on the DVE,
                so scatter the sums across partitions via a small SBUF DMA
                round-trip — except on the tail chains, where ACT is free
                and its (less accurate) reciprocal saves both DMA hops."""
                fc = (h * hd) // P
                po = (h * hd) % P
                q0 = sh * SCW + c0
                av_sb = normp.tile([hd + 1, cw], f32, tag=f"av_sb{cw}")
                if tail:
                    nc.scalar.copy(av_sb[:], avo[:, c0 : c0 + cw])
                else:
                    nc.vector.tensor_copy(av_sb[:], avo[:, c0 : c0 + cw])
                # NOTE: ACT-engine Reciprocal is NOT usable here — its mere
                # presence flips the activation-table selection to one whose
                # Exp variant runs ~20% slower (1336ns vs 1112ns per call,
                # +29us across the kernel).
                recip = normp.tile([1, cw], f32, tag=f"recip{cw}")
                rsh = normp.tile([P, cw // P], f32, tag=f"rsh{cw}")
                nc.sync.dma_start(out=rsh[:], in_=av_sb[hd : hd + 1, :])
                rsh2 = normp.tile([P, cw // P], f32, tag=f"rsh2{cw}")
                nc.vector.reciprocal(rsh2[:], rsh[:])
                nc.sync.dma_start(out=recip[:], in_=rsh2[:])
                bc = normp.tile([hd, cw], f32, tag=f"bc{cw}")
                nc.gpsimd.partition_broadcast(bc[:], recip[:])
                nc.vector.tensor_mul(
                    outT_sb[po : po + hd, fc, q0 : q0 + cw],
                    av_sb[0:hd, :],
                    bc[:],
                )

            def pass_end(avo, h, sh):
                pass_end_q(avo, h, sh, 0, SCW)

            def do_pass(h, sh, mid_kc=None):
                avo = pass_begin()
                pass_blocks(avo, h, sh, range(NT), mid_kc=mid_kc)
                pass_end(avo, h, sh)

            # ---- Phase 1 + first q-half heads 0/1, emission-interleaved ----
            with (
                tc.tile_pool(name="xw", bufs=1) as xw,
                tc.tile_pool(name="p1ps", bufs=2, space="PSUM") as p1ps,
            ):
                xT_r = xw.tile([P, DC, n], bf16)
                wqT_r = xw.tile([P, DC, f], bf16)
                wkT_r = xw.tile([P, DC, f], bf16)
                wvT_r = xw.tile([P, DC, f], bf16)

                # xT streams in COLUMN HALVES: everything up to the first
                # half-pass (Q proj q-half 0, K proj kc 0-7, V tiles nt 0-7,
                # attention kc 0-7) reads only xT[:, 0:SCW], so the first
                # exp fires much earlier than waiting for the full 4MB.
                # wq rides the first half-stream per chunk. All DMAs stay on
                # one queue: the hardware funnels them through a single DMA
                # engine FIFO, so a second issue queue only reorders
                # transfers in front of critical ones.
                for dc in range(DC):
                    nc.sync.dma_start(out=wqT_r[:, dc, :], in_=wqT_c[dc])
                    nc.sync.dma_start(
                        out=xT_r[:, dc, 0:SCW], in_=xT_c[dc][:, 0:SCW]
                    )

                def proj_cols(w_sb, is_k, fc, qcp):
                    # dc-outer accumulation, one sub-stage of 2 held banks
                    # covering moving columns [qcp*QB, (qcp+2)*QB)
                    pss = [
                        p1ps.tile([P, QB], f32, tag="big", name=f"pj{g}")
                        for g in range(2)
                    ]
                    for dc in range(DC):
                        for j in range(2):
                            qc = qcp + j
                            nc.tensor.matmul(
                                pss[j][:],
                                w_sb[:, dc, fc * P : (fc + 1) * P],
                                xT_r[:, dc, qc * QB : (qc + 1) * QB],
                                start=(dc == 0),
                                stop=(dc == DC - 1),
                            )
                    for j in range(2):
                        qc = qcp + j
                        sl = slice(qc * QB, (qc + 1) * QB)
                        if is_k:
                            # rows 0:64 = head 2fc (po=0), rows 64:128 =
                            # head 2fc+1 (po=64); keep row alignment
                            nc.vector.tensor_copy(
                                KTz_sb[0:hd, 2 * fc, sl], pss[j][0:hd, :]
                            )
                            nc.vector.tensor_copy(
                                KTz_sb[hd : 2 * hd, 2 * fc + 1, sl],
                                pss[j][hd : 2 * hd, :],
                            )
                        else:
                            nc.vector.tensor_copy(QT_sb[:, fc, sl], pss[j][:])

                def v_tile(nt):
                    ps = p1ps.tile([P, QB], f32, tag="big", name="vps")
                    for dc in range(DC):
                        nc.tensor.matmul(
                            ps[:, 0:f],
                            xT_r[:, dc, nt * P : (nt + 1) * P],
                            wvT_r[:, dc, :],
                            start=(dc == 0),
                            stop=(dc == DC - 1),
                        )
                    nc.vector.tensor_copy(
                        V_sb[:, nt, :, 0:hd],
                        ps[:, 0:f].rearrange("p (h e) -> p h e", h=hpc),
                    )

                # wk needed right after the first k0 sub-stage; wv by the
                # first v_tile; xT half B by the second half-pass (~22us).
                nc.sync.dma_start(out=wkT_r[:], in_=wkT_p)
                nc.sync.dma_start(out=wvT_r[:], in_=wvT_p)
                for dc in range(DC):
                    nc.sync.dma_start(
                        out=xT_r[:, dc, SCW:n], in_=xT_c[dc][:, SCW:n]
                    )
                woT_sb = outp.tile([P, FC, d], bf16)
                fill_ktz_zeros()

                def wo_half(sh, wo_psum):
                    # output projection for q-half sh (woT stationary, 2
                    # moving q-blocks per weight load; emits partial^T [d, n])
                    q0 = sh * SCW
                    for do in range(d // P):
                        pss = [wo_psum(f"wo{i}") for i in range(SCW // QB)]
                        for fc in range(FC):
                            for qc in range(SCW // QB):
                                nc.tensor.matmul(
                                    pss[qc][:],
                                    woT_sb[:, fc, do * P : (do + 1) * P],
                                    outT_sb[
                                        :, fc, q0 + qc * QB : q0 + (qc + 1) * QB
                                    ],
                                    start=(fc == 0),
                                    stop=(fc == FC - 1),
                                )
                        for qc in range(SCW // QB):
                            ob = wosbp.tile([P, QB], f32, tag="ob")
                            nc.vector.tensor_copy(ob[:], pss[qc][:])
                            nc.sync.dma_start(
                                out=out[
                                    do * P : (do + 1) * P,
                                    q0 + qc * QB : q0 + (qc + 1) * QB,
                                ],
                                in_=ob[:],
                            )

                def wo_q(sh, qh, wo_psum):
                    # 512-wide output-projection chunk for the tail: 2 MMs +
                    # one bounce copy per do-block; copies alternate DVE/ACT
                    # (ACT is done with exps by now)
                    q0 = sh * SCW + qh * QB
                    for do in range(d // P):
                        ps = wo_psum(f"wo{qh}")
                        for fc in range(FC):
                            nc.tensor.matmul(
                                ps[:],
                                woT_sb[:, fc, do * P : (do + 1) * P],
                                outT_sb[:, fc, q0 : q0 + QB],
                                start=(fc == 0),
                                stop=(fc == FC - 1),
                            )
                        ob = wosbp.tile([P, QB], f32, tag="ob")
                        if do % 2 == 1:
                            nc.scalar.copy(ob[:], ps[:])
                        else:
                            nc.vector.tensor_copy(ob[:], ps[:])
                        nc.sync.dma_start(
                            out=out[do * P : (do + 1) * P, q0 : q0 + QB],
                            in_=ob[:],
                        )

                # Emission order = scheduling priority. Minimal chain to the
                # first exp: QT cols of the first q-half, then K^T in column
                # sub-stages interleaved with head 0's pass blocks (V tiles
                # interleaved per k-chunk they feed). Later projections are
                # emitted after the passes they should yield priority to, so
                # they fill the PE's ACT-paced slack.
                proj_cols(wqT_r, False, 0, 0)  # QT fc0 cols 0:1024 (q-half 0)
                avo0 = pass_begin()
                proj_cols(wkT_r, True, 0, 0)   # KTz fc0 cols 0:1024 (kc 0..7)
                pass_blocks(avo0, 0, 0, range(0, NT // 2), mid_kc=v_tile)
                proj_cols(wkT_r, True, 0, 2)   # KTz fc0 cols 1024:2048
                pass_blocks(avo0, 0, 0, range(NT // 2, NT), mid_kc=v_tile)
                pass_end(avo0, 0, 0)
                do_pass(1, 0)
                proj_cols(wqT_r, False, 0, 2)  # QT fc0 cols for q-half 1
                do_pass(0, 1)
                do_pass(1, 1)
                proj_cols(wqT_r, False, 1, 0)
                proj_cols(wqT_r, False, 1, 2)
                proj_cols(wkT_r, True, 1, 0)
                proj_cols(wkT_r, True, 1, 2)

            # ---- remaining passes + per-q-half output projection ----
            with (
                tc.tile_pool(name="wops", bufs=2, space="PSUM") as wopsp,
            ):
                nc.sync.dma_start(out=woT_sb[:], in_=woT_p)

                def wo_psum(name):
                    return wopsp.tile([P, QB], f32, tag="wops", name=name)

                do_pass(2, 0)
                do_pass(3, 0)
                wo_half(0, wo_psum)
                do_pass(2, 1)
                # final head: normalize + project in 512-wide chunks so the
                # first wo chunk overlaps the second chunk's normalize chain
                # instead of idling the PE for the whole pass_end latency
                avo3 = pass_begin()
                pass_blocks(avo3, 3, 1, range(NT))
                pass_end_q(avo3, 3, 1, 0, QB, tail=True)
                pass_end_q(avo3, 3, 1, QB, QB, tail=True)
                # PE warm-keeper: the normalize chains above take ~6us of
                # cross-engine latency during which the PE has nothing
                # runnable; an idle PE drops out of its boosted p-state and
                # the 32 wo matmuls below then run at half clock. Burn the
                # wait on discardable matmuls (results overwritten by the
                # start=True wo accumulations reusing the same slots).
                # operands must have NO pending writers (KTz/QT were finished
                # long ago) or the dep tracker chains the warm MMs behind the
                # normalize they are meant to shadow
                for w in range(20):
                    wps = wo_psum("warm")
                    nc.tensor.matmul(
                        wps[:],
                        KTz_sb[:, 0, 0:P],
                        QT_sb[:, 0, 0:QB],
                        start=True,
                        stop=True,
                    )
                wo_q(1, 0, wo_psum)
                wo_q(1, 1, wo_psum)
    nc.finalize()
    return nc


def make_in_maps(x, Wq, Wk, Wv, Wo):
    """Shard full inputs into per-core DRAM parameter maps (bf16)."""
    import ml_dtypes

    bf = ml_dtypes.bfloat16
    x = np.asarray(x, dtype=np.float32)
    Wq = np.asarray(Wq, dtype=np.float32)
    Wk = np.asarray(Wk, dtype=np.float32)
    Wv = np.asarray(Wv, dtype=np.float32)
    Wo = np.asarray(Wo, dtype=np.float32)
    xTs = [np.ascontiguousarray(x[b].T).astype(bf) for b in range(B)]
    WqT, WkT, WvT = Wq.T, Wk.T, Wv.T
    in_maps = []
    for c in range(N_CORES):
        b, g = c // (N_CORES // B), c % (N_CORES // B)
        fs = slice(g * F, (g + 1) * F)
        in_maps.append(
            {
                "xT": xTs[b],
                "wqT": np.ascontiguousarray(WqT[:, fs]).astype(bf),
                "wkT": np.ascontiguousarray(WkT[:, fs]).astype(bf),
                "wvT": np.ascontiguousarray(WvT[:, fs]).astype(bf),
                "woT": np.ascontiguousarray(Wo[:, fs].T).astype(bf),
            }
        )
    return in_maps


def _dedupe_ldweights(nc):
    """Drop PE weight reloads of the already-loaded stationary.

    Tile legalization splits every 2-byte matmul into InstLdweights +
    InstMatmult(ldweights=False) with no dedup, so back-to-back matmuls
    sharing a stationary reload it redundantly (~100ns each on the PE
    pipeline). Post-finalize the per-block instruction order is final;
    walk it tracking the loaded stationary (memref/offset/ap/transpose/
    perf_mode) and delete an InstLdweights that matches it. Safety:
    matmuls here never self-load (checked), non-PE instructions don't
    touch the PE array, and a dropped LDW must carry no semaphore waits
    or updates (else it is kept).
    """
    f = nc.m.functions[0]
    dropped = 0
    for bb in f.blocks:
        insts = bb.instructions
        loaded = None
        drop = []
        for pos, i in enumerate(insts):
            if str(getattr(i, "engine", "")) != "EngineType.PE":
                continue
            tn = type(i).__name__
            if tn == "InstLdweights":
                ap = i.ins[0]
                key = (
                    ap.memref,
                    ap.offset,
                    str(ap.ap),
                    str(i.is_transpose),
                    str(i.perf_mode),
                )
                si = i.sync_info
                clean = si is None or (not list(si.on_wait) and not list(si.on_update))
                if key == loaded and clean:
                    drop.append(pos)
                else:
                    loaded = key
            elif tn == "InstMatmult":
                if i.ldweights is not False:
                    loaded = None  # self-loading matmul clobbers the array
            elif tn in ("InstEventSemaphore", "InstDrain", "InstNop"):
                pass  # sequencer-only; weight array untouched
            else:
                loaded = None
        for pos in reversed(drop):
            insts.remove(insts[pos])
        dropped += len(drop)
    return dropped


def _force_act_table(nc, set_id):
    """Rewrite the act-table load(s) to a specific act_func_sets index.
    Exp implementations differ per table; the auto-picked table is not
    necessarily the fastest one that covers {Exp, Copy}."""
    n = 0
    for bb in nc.m.functions[0].blocks:
        for i in bb.instructions:
            if type(i).__name__ == "InstLoadActFuncSet":
                i.act_func_set_id = set_id
                n += 1
    return n


_NC_CACHE = {}


def _enable_ldw_opt():
    """Flip walrus --enable-ldw-opt to true: consecutive matmuls sharing a
    stationary operand skip the redundant LDWEIGHTS reload."""
    import concourse.bass_utils as bu

    if getattr(bu, "_ldw_opt_patched", False):
        return
    orig = bu.run_command

    def patched(argv, **kw):
        argv = [
            "--enable-ldw-opt=true" if a == "--enable-ldw-opt=false" else a
            for a in argv
        ]
        return orig(argv, **kw)

    bu.run_command = patched
    bu._ldw_opt_patched = True


def run(x, Wq, Wk, Wv, Wo, trace=False):
    from concourse.bass_utils import run_bass_kernel_spmd

    # NOTE: the walrus --enable-ldw-opt patch is f32r-only: for 2-byte
    # dtypes tile legalization emits explicit InstLdweights (deduped
    # there), and walrus rejects ldw-opt on explicit Ldweights.
    import os

    if "nc" not in _NC_CACHE:
        nc = build_nc()
        _dedupe_ldweights(nc)
        tbl = os.environ.get("ACT_TABLE")
        if tbl is not None:
            _force_act_table(nc, int(tbl))
        _NC_CACHE["nc"] = nc
    nc = _NC_CACHE["nc"]
    in_maps = make_in_maps(x, Wq, Wk, Wv, Wo)
    res = run_bass_kernel_spmd(nc, in_maps, core_ids=list(range(N_CORES)), trace=trace)
    parts = [np.asarray(res.results[i]["out"]) for i in range(N_CORES)]
    gpb = N_CORES // B
    # per-core partials are transposed [d, n]: sum the group, then untranspose
    full = np.stack(
        [
            sum(parts[b * gpb + 1 : (b + 1) * gpb], parts[b * gpb]).T
            for b in range(B)
        ]
    )
    return np.ascontiguousarray(full, dtype=np.float32), res


def kernel(x, Wq, bq, Wk, bk, Wv, bv, Wo, bo):
    full, _ = run(x, Wq, Wk, Wv, Wo)
    return full



# revision 55
# speedup vs baseline: 1.0120x; 1.0120x over previous
"""Multi-head attention kernel for 8 TRN2 NeuronCores.

Problem: b=2, n=2048, d=1024, heads=16, hd=64.
  q/k/v = x @ W{q,k,v}.T (+ zero bias)
  per head: softmax(q k^T / sqrt(d)) @ v
  out = concat @ Wo.T (+ zero bias)

Sharding (8 cores): data-parallel over batch (2) x tensor-parallel over
heads (16 heads -> 4 groups of 4). Core c handles batch c//4, heads
4*(c%4) .. 4*(c%4)+3 (feature slice of 256 columns). Wo is applied
row-parallel: each core emits a partial output; the host sums the 4
partials per batch (and untransposes). No collectives needed.

All matmuls run in float32r (TF32-like: ~1.5e-4 rel err on a K=1024
contraction, 4x the fp32 PE rate, full rate only when the moving free
dim is >=256). Raw fp32 bits are DMA'd directly into f32r tiles
(measured identical to an explicit rounding pass). On-chip f32r
operands (Q^T/K^T/V/P^T/out^T) are written by rounding copy/activation
producers as the walrus verifier requires.

Key structure decisions (all measured on HW):
 - everything is pre-transposed on the host so the kernel needs zero
   on-device transposes: xT (d,n), wqT/wkT/wvT (d,256), woT (256,d).
 - Q^T/K^T [feat, n] via dc-outer accumulation streaming behind the
   xT DMA; V in natural [n, feat] layout with a ones column appended
   (the ones column accumulates the softmax denominators during AV).
 - K^T is stored zero-padded per head to a full 128-row stationary:
   K=64 matmuls run at 2 cyc/row and read as low PE activity (HAM
   clock-gates to half speed); zero-padded K=128 runs at 1 cyc/row.
 - scores^T[k, q] (PE) -> exp via ScalarE reading 2 PSUM banks per
   call (the ACT engine is the pacing floor: n*n*heads/core exps at 1
   elem/cycle/lane) -> AV accumulates V_aug^T . P^T in PSUM [65, q].
 - passes are (q-half, head)-ordered and their emission is interleaved
   with the fc=1 projections so the PE fills ACT-paced slack; each
   q-half's output projection runs in the next half's ACT shadow.
 - normalize: copy avo out of PSUM fast (frees the accumulator), then
   reciprocal in a [128, 8] partition-scattered layout (a [1, 1024]
   row reciprocal is single-lane and 60x slower), partition_broadcast
   on GpSimd, multiply on DVE.
 - output projection keeps woT stationary (2 moving blocks per weight
   load) and emits the partial TRANSPOSED [d, n]; the host untransposes.

Biases are structurally zero in this problem spec and are skipped.
"""

import numpy as np

HEADS = 16
D = 1024
N = 2048
B = 2
N_CORES = 8
HPC = HEADS // (N_CORES // B)  # heads per core = 4
HD = D // HEADS                # 64
F = HPC * HD                   # 256 features per core
P = 128


def build_nc(n=N, d=D, hpc=HPC, hd=HD):
    """Build the per-core Bass program (SPMD: same program on all 8 cores)."""
    import concourse.bass as bass
    import concourse.tile as tile
    from concourse import bacc, mybir

    f32 = mybir.dt.float32
    bf16 = mybir.dt.bfloat16
    f = hpc * hd            # per-core feature count (256)
    FC = f // P             # feature chunks (2)
    DC = d // P             # contraction chunks over d (8)
    NT = n // P             # n tiles / k chunks (16)
    QB = min(512, n)        # matmul moving block
    SCW = min(1024, n)      # scores psum width (2 banks)
    NSC = n // SCW          # q-halves
    scale = 1.0 / float(np.sqrt(np.float32(d)))

    nc = bacc.Bacc("TRN2")

    xT = nc.declare_dram_parameter("xT", [d, n], bf16, isOutput=False)
    wqT = nc.declare_dram_parameter("wqT", [d, f], bf16, isOutput=False)
    wkT = nc.declare_dram_parameter("wkT", [d, f], bf16, isOutput=False)
    wvT = nc.declare_dram_parameter("wvT", [d, f], bf16, isOutput=False)
    woT = nc.declare_dram_parameter("woT", [f, d], bf16, isOutput=False)
    out = nc.declare_dram_parameter("out", [d, n], f32, isOutput=True)

    xT_c = xT.rearrange("(c p) n -> c p n", p=P)
    wqT_c = wqT.rearrange("(c p) f -> c p f", p=P)
    wkT_p = wkT.rearrange("(c p) f -> p c f", p=P)
    wvT_p = wvT.rearrange("(c p) f -> p c f", p=P)
    woT_p = woT.rearrange("(c p) n -> p c n", p=P)

    with tile.TileContext(nc) as tc:
        with (
            tc.tile_pool(name="qkv", bufs=1) as qkv,
            tc.tile_pool(name="outT", bufs=1) as outp,
            tc.tile_pool(name="wosb", bufs=4) as wosbp,
            # phase-2 pools created before the phase-1 pools so their
            # SBUF/PSUM ranges are disjoint: early heads' attention overlaps
            # the fc=1 projections with no pool-reuse serialization
            tc.tile_pool(name="pt", bufs=2) as ptp,
            tc.tile_pool(name="norm", bufs=2) as normp,
            tc.tile_pool(name="scps", bufs=2, space="PSUM") as scps,
            tc.tile_pool(name="avps", bufs=1, space="PSUM") as avps,
        ):
            QT_sb = qkv.tile([P, FC, n], bf16)
            # per-head K^T, zero-padded to a full 128-row stationary (head h
            # occupies partition rows po..po+hd, matching its rows in QT)
            KTz_sb = qkv.tile([P, hpc, n], bf16)
            V_sb = qkv.tile([P, NT, hpc, hd + 1], bf16)
            outT_sb = outp.tile([P, FC, n], bf16)
            # ones column of V_aug via broadcast copy
            ones_c = outp.tile([P, 1], f32)
            nc.vector.memset(ones_c[:], 1.0)
            nc.vector.tensor_copy(
                V_sb[:, :, :, hd : hd + 1],
                ones_c.to_broadcast([P, NT, hpc, 1]),
            )
            def fill_ktz_zeros():
                # KTz zero padding: only the COMPLEMENT rows of each head's
                # stationary stay zero (head 2fc occupies rows 0:hd, head
                # 2fc+1 rows hd:2hd). Zeroing just those keeps the fill
                # disjoint from the K^T copies (no WAR serialization); it
                # runs on the scalar engine, emitted AFTER the scalar-queue
                # DMA issues so it doesn't delay them.
                nc.scalar.memzero(KTz_sb[hd : 2 * hd, 0:hpc:2, :])
                nc.scalar.memzero(KTz_sb[0:hd, 1:hpc:2, :])

            def pass_begin():
                return avps.tile([hd + 1, SCW], f32, tag="avo", name="avo")

            def pass_blocks(avo, h, sh, kcs, mid_kc=None):
                """scores^T -> exp -> AV accumulate for k-chunks `kcs`.
                mid_kc (e.g. the V tile build feeding this kc's AV) emits
                BETWEEN scores and AV so the exp starts as soon as the
                scores land instead of queueing behind the filler's PE
                work."""
                fc = (h * hd) // P
                q0 = sh * SCW
                for kc in kcs:
                    sc = scps.tile([P, SCW], f32, tag="sc")
                    for qc in range(SCW // QB):
                        nc.tensor.matmul(
                            sc[:, qc * QB : (qc + 1) * QB],
                            KTz_sb[:, h, kc * P : (kc + 1) * P],
                            QT_sb[:, fc, q0 + qc * QB : q0 + (qc + 1) * QB],
                            start=True,
                            stop=True,
                        )
                    pt = ptp.tile([P, SCW], bf16, tag="pt")
                    nc.scalar.activation(
                        pt[:], sc[:], mybir.ActivationFunctionType.Exp,
                        scale=scale,
                    )
                    if mid_kc is not None:
                        mid_kc(kc)
                    for qc in range(SCW // QB):
                        nc.tensor.matmul(
                            avo[:, qc * QB : (qc + 1) * QB],
                            V_sb[:, kc, h, :],
                            pt[:, qc * QB : (qc + 1) * QB],
                            start=(kc == 0),
                            stop=(kc == NT - 1),
                        )

            def act_reciprocal(out_ap, in_ap):
                """ACT-engine reciprocal, bypassing the bass API's accuracy
                guard (tolerance here is 2e-2; the ACT table recip's error is
                orders below that). Single instruction, no partition scatter
                needed — ACT paces on free size, not partition count."""
                return nc.scalar.add_instruction(
                    mybir.InstActivation(
                        name=nc.get_next_instruction_name(),
                        func=mybir.ActivationFunctionType.Reciprocal,
                        ins=[
                            nc.scalar.lower_ap(in_ap),
                            mybir.ImmediateValue(
                                dtype=mybir.dt.float32, value=0.0
                            ),
                            mybir.ImmediateValue(
                                dtype=mybir.dt.float32, value=1.0
                            ),
                            mybir.ImmediateValue(
                                dtype=mybir.dt.float32, value=0.0
                            ),
                        ],
                        outs=[nc.scalar.lower_ap(out_ap)],
                    )
                )

            def pass_end_q(avo, h, sh, c0, cw, tail=False):
                """Normalize columns [c0, c0+cw) of avo: rows 0..hd-1 divided
                by row hd (the softmax sums). The avo drain copy runs on the
                otherwise-idle gpsimd so it never queues behind DVE work
                (stalling the next pass's first AV on the avo banks).
                reciprocal is single-lane-slow on a [1, cw] row on the DVE,
                so scatter the sums across partitions via a small SBUF DMA
                round-trip — except on the tail chains, where ACT is free
                and its (less accurate) reciprocal saves both DMA hops."""
                fc = (h * hd) // P
                po = (h * hd) % P
                q0 = sh * SCW + c0
                av_sb = normp.tile([hd + 1, cw], f32, tag=f"av_sb{cw}")
                if tail:
                    nc.scalar.copy(av_sb[:], avo[:, c0 : c0 + cw])
                else:
                    nc.vector.tensor_copy(av_sb[:], avo[:, c0 : c0 + cw])
                # NOTE: ACT-engine Reciprocal is NOT usable here — its mere
                # presence flips the activation-table selection to one whose
                # Exp variant runs ~20% slower (1336ns vs 1112ns per call,
                # +29us across the kernel).
                recip = normp.tile([1, cw], f32, tag=f"recip{cw}")
                rsh = normp.tile([P, cw // P], f32, tag=f"rsh{cw}")
                nc.sync.dma_start(out=rsh[:], in_=av_sb[hd : hd + 1, :])
                rsh2 = normp.tile([P, cw // P], f32, tag=f"rsh2{cw}")
                nc.vector.reciprocal(rsh2[:], rsh[:])
                nc.sync.dma_start(out=recip[:], in_=rsh2[:])
                bc = normp.tile([hd, cw], f32, tag=f"bc{cw}")
                nc.gpsimd.partition_broadcast(bc[:], recip[:])
                nc.vector.tensor_mul(
                    outT_sb[po : po + hd, fc, q0 : q0 + cw],
                    av_sb[0:hd, :],
                    bc[:],
                )

            def pass_end(avo, h, sh):
                pass_end_q(avo, h, sh, 0, SCW)

            def do_pass(h, sh, mid_kc=None):
                avo = pass_begin()
                pass_blocks(avo, h, sh, range(NT), mid_kc=mid_kc)
                pass_end(avo, h, sh)

            # ---- Phase 1 + first q-half heads 0/1, emission-interleaved ----
            with (
                tc.tile_pool(name="xw", bufs=1) as xw,
                tc.tile_pool(name="p1ps", bufs=2, space="PSUM") as p1ps,
            ):
                xT_r = xw.tile([P, DC, n], bf16)
                wqT_r = xw.tile([P, DC, f], bf16)
                wkT_r = xw.tile([P, DC, f], bf16)
                wvT_r = xw.tile([P, DC, f], bf16)

                # xT streams in COLUMN HALVES: everything up to the first
                # half-pass (Q proj q-half 0, K proj kc 0-7, V tiles nt 0-7,
                # attention kc 0-7) reads only xT[:, 0:SCW], so the first
                # exp fires much earlier than waiting for the full 4MB.
                # wq rides the first half-stream per chunk. All DMAs stay on
                # one queue: the hardware funnels them through a single DMA
                # engine FIFO, so a second issue queue only reorders
                # transfers in front of critical ones.
                for dc in range(DC):
                    nc.sync.dma_start(out=wqT_r[:, dc, :], in_=wqT_c[dc])
                    nc.sync.dma_start(
                        out=xT_r[:, dc, 0:SCW], in_=xT_c[dc][:, 0:SCW]
                    )

                def proj_cols(w_sb, is_k, fc, qcp):
                    # dc-outer accumulation, one sub-stage of 2 held banks
                    # covering moving columns [qcp*QB, (qcp+2)*QB)
                    pss = [
                        p1ps.tile([P, QB], f32, tag="big", name=f"pj{g}")
                        for g in range(2)
                    ]
                    for dc in range(DC):
                        for j in range(2):
                            qc = qcp + j
                            nc.tensor.matmul(
                                pss[j][:],
                                w_sb[:, dc, fc * P : (fc + 1) * P],
                                xT_r[:, dc, qc * QB : (qc + 1) * QB],
                                start=(dc == 0),
                                stop=(dc == DC - 1),
                            )
                    for j in range(2):
                        qc = qcp + j
                        sl = slice(qc * QB, (qc + 1) * QB)
                        if is_k:
                            # rows 0:64 = head 2fc (po=0), rows 64:128 =
                            # head 2fc+1 (po=64); keep row alignment
                            nc.vector.tensor_copy(
                                KTz_sb[0:hd, 2 * fc, sl], pss[j][0:hd, :]
                            )
                            nc.vector.tensor_copy(
                                KTz_sb[hd : 2 * hd, 2 * fc + 1, sl],
                                pss[j][hd : 2 * hd, :],
                            )
                        else:
                            nc.vector.tensor_copy(QT_sb[:, fc, sl], pss[j][:])

                def v_tile(nt):
                    ps = p1ps.tile([P, QB], f32, tag="big", name="vps")
                    for dc in range(DC):
                        nc.tensor.matmul(
                            ps[:, 0:f],
                            xT_r[:, dc, nt * P : (nt + 1) * P],
                            wvT_r[:, dc, :],
                            start=(dc == 0),
                            stop=(dc == DC - 1),
                        )
                    nc.vector.tensor_copy(
                        V_sb[:, nt, :, 0:hd],
                        ps[:, 0:f].rearrange("p (h e) -> p h e", h=hpc),
                    )

                # wk needed right after the first k0 sub-stage; wv by the
                # first v_tile; xT half B by the second half-pass (~22us).
                nc.sync.dma_start(out=wkT_r[:], in_=wkT_p)
                nc.sync.dma_start(out=wvT_r[:], in_=wvT_p)
                for dc in range(DC):
                    nc.sync.dma_start(
                        out=xT_r[:, dc, SCW:n], in_=xT_c[dc][:, SCW:n]
                    )
                woT_sb = outp.tile([P, FC, d], bf16)
                fill_ktz_zeros()

                def wo_half(sh, wo_psum):
                    # output projection for q-half sh (woT stationary, 2
                    # moving q-blocks per weight load; emits partial^T [d, n])
                    q0 = sh * SCW
                    for do in range(d // P):
                        pss = [wo_psum(f"wo{i}") for i in range(SCW // QB)]
                        for fc in range(FC):
                            for qc in range(SCW // QB):
                                nc.tensor.matmul(
                                    pss[qc][:],
                                    woT_sb[:, fc, do * P : (do + 1) * P],
                                    outT_sb[
                                        :, fc, q0 + qc * QB : q0 + (qc + 1) * QB
                                    ],
                                    start=(fc == 0),
                                    stop=(fc == FC - 1),
                                )
                        for qc in range(SCW // QB):
                            ob = wosbp.tile([P, QB], f32, tag="ob")
                            nc.vector.tensor_copy(ob[:], pss[qc][:])
                            nc.sync.dma_start(
                                out=out[
                                    do * P : (do + 1) * P,
                                    q0 + qc * QB : q0 + (qc + 1) * QB,
                                ],
                                in_=ob[:],
                            )

                def wo_q(sh, qh, wo_psum):
                    # 512-wide output-projection chunk for the tail: 2 MMs +
                    # one bounce copy per do-block; copies alternate DVE/ACT
                    # (ACT is done with exps by now)
                    q0 = sh * SCW + qh * QB
                    for do in range(d // P):
                        ps = wo_psum(f"wo{qh}")
                        for fc in range(FC):
                            nc.tensor.matmul(
                                ps[:],
                                woT_sb[:, fc, do * P : (do + 1) * P],
                                outT_sb[:, fc, q0 : q0 + QB],
                                start=(fc == 0),
                                stop=(fc == FC - 1),
                            )
                        ob = wosbp.tile([P, QB], f32, tag="ob")
                        if do % 2 == 1:
                            nc.scalar.copy(ob[:], ps[:])
                        else:
                            nc.vector.tensor_copy(ob[:], ps[:])
                        nc.sync.dma_start(
                            out=out[do * P : (do + 1) * P, q0 : q0 + QB],
                            in_=ob[:],
                        )

                # Emission order = scheduling priority. Minimal chain to the
                # first exp: QT cols of the first q-half, then K^T in column
                # sub-stages interleaved with head 0's pass blocks (V tiles
                # interleaved per k-chunk they feed). Later projections are
                # emitted after the passes they should yield priority to, so
                # they fill the PE's ACT-paced slack.
                proj_cols(wqT_r, False, 0, 0)  # QT fc0 cols 0:1024 (q-half 0)
                avo0 = pass_begin()
                proj_cols(wkT_r, True, 0, 0)   # KTz fc0 cols 0:1024 (kc 0..7)
                pass_blocks(avo0, 0, 0, range(0, NT // 2), mid_kc=v_tile)
                proj_cols(wkT_r, True, 0, 2)   # KTz fc0 cols 1024:2048
                pass_blocks(avo0, 0, 0, range(NT // 2, NT), mid_kc=v_tile)
                pass_end(avo0, 0, 0)
                do_pass(1, 0)
                proj_cols(wqT_r, False, 0, 2)  # QT fc0 cols for q-half 1
                do_pass(0, 1)
                do_pass(1, 1)
                proj_cols(wqT_r, False, 1, 0)
                proj_cols(wqT_r, False, 1, 2)
                proj_cols(wkT_r, True, 1, 0)
                proj_cols(wkT_r, True, 1, 2)

            # ---- remaining passes + per-q-half output projection ----
            with (
                tc.tile_pool(name="wops", bufs=2, space="PSUM") as wopsp,
            ):
                nc.sync.dma_start(out=woT_sb[:], in_=woT_p)

                def wo_psum(name):
                    return wopsp.tile([P, QB], f32, tag="wops", name=name)

                do_pass(2, 0)
                do_pass(3, 0)
                wo_half(0, wo_psum)
                do_pass(2, 1)
                # final head: normalize + project in 512-wide chunks so the
                # first wo chunk overlaps the second chunk's normalize chain
                # instead of idling the PE for the whole pass_end latency
                avo3 = pass_begin()
                pass_blocks(avo3, 3, 1, range(NT))
                pass_end_q(avo3, 3, 1, 0, QB, tail=True)
                pass_end_q(avo3, 3, 1, QB, QB, tail=True)
                # PE warm-keeper: the normalize chains above take ~6us of
                # cross-engine latency during which the PE has nothing
                # runnable; an idle PE drops out of its boosted p-state and
                # the 32 wo matmuls below then run at half clock. Burn the
                # wait on discardable matmuls (results overwritten by the
                # start=True wo accumulations reusing the same slots).
                # operands and psum slots must have NO pending consumers
                # (KTz/QT and the sc slots' exps finished long ago) or the
                # dep tracker chains the warm MMs behind the normalize they
                # are meant to shadow
                for w in range(20):
                    wps = scps.tile([P, SCW], f32, tag="sc")
                    nc.tensor.matmul(
                        wps[:, 0:QB],
                        KTz_sb[:, 0, 0:P],
                        QT_sb[:, 0, 0:QB],
                        start=True,
                        stop=True,
                    )
                wo_q(1, 0, wo_psum)
                wo_q(1, 1, wo_psum)
    nc.finalize()
    return nc


def make_in_maps(x, Wq, Wk, Wv, Wo):
    """Shard full inputs into per-core DRAM parameter maps (bf16)."""
    import ml_dtypes

    bf = ml_dtypes.bfloat16
    x = np.asarray(x, dtype=np.float32)
    Wq = np.asarray(Wq, dtype=np.float32)
    Wk = np.asarray(Wk, dtype=np.float32)
    Wv = np.asarray(Wv, dtype=np.float32)
    Wo = np.asarray(Wo, dtype=np.float32)
    xTs = [np.ascontiguousarray(x[b].T).astype(bf) for b in range(B)]
    WqT, WkT, WvT = Wq.T, Wk.T, Wv.T
    in_maps = []
    for c in range(N_CORES):
        b, g = c // (N_CORES // B), c % (N_CORES // B)
        fs = slice(g * F, (g + 1) * F)
        in_maps.append(
            {
                "xT": xTs[b],
                "wqT": np.ascontiguousarray(WqT[:, fs]).astype(bf),
                "wkT": np.ascontiguousarray(WkT[:, fs]).astype(bf),
                "wvT": np.ascontiguousarray(WvT[:, fs]).astype(bf),
                "woT": np.ascontiguousarray(Wo[:, fs].T).astype(bf),
            }
        )
    return in_maps


def _dedupe_ldweights(nc):
    """Drop PE weight reloads of the already-loaded stationary.

    Tile legalization splits every 2-byte matmul into InstLdweights +
    InstMatmult(ldweights=False) with no dedup, so back-to-back matmuls
    sharing a stationary reload it redundantly (~100ns each on the PE
    pipeline). Post-finalize the per-block instruction order is final;
    walk it tracking the loaded stationary (memref/offset/ap/transpose/
    perf_mode) and delete an InstLdweights that matches it. Safety:
    matmuls here never self-load (checked), non-PE instructions don't
    touch the PE array, and a dropped LDW must carry no semaphore waits
    or updates (else it is kept).
    """
    f = nc.m.functions[0]
    dropped = 0
    for bb in f.blocks:
        insts = bb.instructions
        loaded = None
        drop = []
        for pos, i in enumerate(insts):
            if str(getattr(i, "engine", "")) != "EngineType.PE":
                continue
            tn = type(i).__name__
            if tn == "InstLdweights":
                ap = i.ins[0]
                key = (
                    ap.memref,
                    ap.offset,
                    str(ap.ap),
                    str(i.is_transpose),
                    str(i.perf_mode),
                )
                si = i.sync_info
                clean = si is None or (not list(si.on_wait) and not list(si.on_update))
                if key == loaded and clean:
                    drop.append(pos)
                else:
                    loaded = key
            elif tn == "InstMatmult":
                if i.ldweights is not False:
                    loaded = None  # self-loading matmul clobbers the array
            elif tn in ("InstEventSemaphore", "InstDrain", "InstNop"):
                pass  # sequencer-only; weight array untouched
            else:
                loaded = None
        for pos in reversed(drop):
            insts.remove(insts[pos])
        dropped += len(drop)
    return dropped


def _force_act_table(nc, set_id):
    """Rewrite the act-table load(s) to a specific act_func_sets index.
    Exp implementations differ per table; the auto-picked table is not
    necessarily the fastest one that covers {Exp, Copy}."""
    n = 0
    for bb in nc.m.functions[0].blocks:
        for i in bb.instructions:
            if type(i).__name__ == "InstLoadActFuncSet":
                i.act_func_set_id = set_id
                n += 1
    return n


_NC_CACHE = {}


def _enable_ldw_opt():
    """Flip walrus --enable-ldw-opt to true: consecutive matmuls sharing a
    stationary operand skip the redundant LDWEIGHTS reload."""
    import concourse.bass_utils as bu

    if getattr(bu, "_ldw_opt_patched", False):
        return
    orig = bu.run_command

    def patched(argv, **kw):
        argv = [
            "--enable-ldw-opt=true" if a == "--enable-ldw-opt=false" else a
            for a in argv
        ]
        return orig(argv, **kw)

    bu.run_command = patched
    bu._ldw_opt_patched = True


def run(x, Wq, Wk, Wv, Wo, trace=False):
    from concourse.bass_utils import run_bass_kernel_spmd

    # NOTE: the walrus --enable-ldw-opt patch is f32r-only: for 2-byte
    # dtypes tile legalization emits explicit InstLdweights (deduped
    # there), and walrus rejects ldw-opt on explicit Ldweights.
    import os

    if "nc" not in _NC_CACHE:
        nc = build_nc()
        _dedupe_ldweights(nc)
        tbl = os.environ.get("ACT_TABLE")
        if tbl is not None:
            _force_act_table(nc, int(tbl))
        _NC_CACHE["nc"] = nc
    nc = _NC_CACHE["nc"]
    in_maps = make_in_maps(x, Wq, Wk, Wv, Wo)
    res = run_bass_kernel_spmd(nc, in_maps, core_ids=list(range(N_CORES)), trace=trace)
    parts = [np.asarray(res.results[i]["out"]) for i in range(N_CORES)]
    gpb = N_CORES // B
    # per-core partials are transposed [d, n]: sum the group, then untranspose
    full = np.stack(
        [
            sum(parts[b * gpb + 1 : (b + 1) * gpb], parts[b * gpb]).T
            for b in range(B)
        ]
    )
    return np.ascontiguousarray(full, dtype=np.float32), res


def kernel(x, Wq, bq, Wk, bk, Wv, bv, Wo, bo):
    full, _ = run(x, Wq, Wk, Wv, Wo)
    return full

